# revision 6
# baseline (speedup 1.0000x reference)
"""AdaptiveSparseAttention Trainium2 kernel (8-core head-parallel).

Problem: B=1, H=16, S=2048, D=128 fp32, causal attention with an adaptive
block mask: mean-pool Q/K per 64-block, softmax block scores, keep the
minimal top-p (0.95) set of key blocks per query block (plus diagonal).

Sharding: 2 heads per NeuronCore, fully local (no collectives).

Per-head algorithm on device:
  - q,k loaded as 16 natural [128,128] f32 chunks, PE-transposed to
    qT/kT [D=128, S=2048]; block sums (for the mask) are computed in f32
    by segmented reduction over the transposed psum tiles; qT/kT are
    stored bf16 in SBUF for the main matmuls; kT gets the per-head mean
    subtracted (smooth-k) in place.
  - block-score pipeline (all f32, 32x32): bl = qb@kb^T * scale/4096 with
    causal -1e30 mask, softmax, then keep[i,j] = (sum of probs strictly
    greater than p_ij) < 0.95, AND causal, OR diagonal.  This reproduces
    the reference argsort/cumsum construction exactly (no ties occur).
  - flash-style attention with *transposed* logits: LT[kj, qi] tiles
    computed as kT_s.T @ qT on PE (bf16), block mask added via a second
    rank-32 matmul (indicator @ expanded -1e9 keep rows, float32r),
    token-level causal handled by one [128,128] triangular DVE add on the
    diagonal tile.  exp via ScalarE with scale=1/sqrt(D), bias=-SHIFT
    (constant shift instead of row max; inputs are N(0,1) so logits are
    bounded), output PT in bf16.  P@V accumulates [128,129] psum tiles
    (ones column appended to V gives the softmax denominator), then a
    reciprocal + per-partition scale finalizes each 128-row output tile.
"""

import math
import os
import threading

import numpy as np

_B, _H, _S, _D = 1, 16, 2048, 128
_NCORES = 8
_HLOC = _H // _NCORES  # heads per core
_BLK = 64
_NB = _S // _BLK       # 32 key/query blocks
_TAU = 0.95
_SCALE = 1.0 / math.sqrt(_D)
_SHIFT = 9.0           # constant softmax shift; |scaled logits| < ~6
_BIGM = 1.0e9          # additive mask magnitude (pre-scale)
_NEG_BL = -1.0e30      # block-logit causal mask value (matches reference)

_NCHUNK = _S // 128    # 16 sequence chunks of 128
_NGRP = _S // 512      # 4 query groups of 512


def _build_head(nc, tc, pools, consts, q_d, k_d, v_d, out_d, h, mybir, bass):
    """Emit IR for one head h."""
    f32 = mybir.dt.float32
    bf16 = mybir.dt.bfloat16
    AF = mybir.ActivationFunctionType
    OP = mybir.AluOpType
    AX = mybir.AxisListType

    natp = pools["natp"]
    psM = pools["psM"]  # shared 1-bank psum slots: transposes, bl, AV accums
    psP = pools["psP"]
    big = pools["big"]
    sm = pools["sm"]
    ptp = pools["ptp"]
    outp = pools["outp"]

    ident = consts["ident"]          # [128,128] f32 identity
    indall = consts["indall"]        # [32, 2048] bf16 block indicator
    tri128 = consts["tri128"]        # [128,128] f32 0 / -BIGM upper tri
    causal_add = consts["causal_add"]  # [32,32] 0 lower / -1e30 upper
    causal01 = consts["causal01"]    # [32,32] 1 lower / 0 upper
    eye01 = consts["eye01"]          # [32,32] identity
    nshift = consts["nshift"]        # [128,1] f32 = -SHIFT

    # ---- persistent per-head SBUF tensors ----
    qT = big.tile([128, _S], bf16, tag="qT")       # [D, S] bf16
    kT = big.tile([128, _S], bf16, tag="kT")       # [D, S] bf16 (k, then k_s)
    vb = big.tile([128, _NCHUNK * 129], bf16, tag="vb")  # V chunks + ones col
    qbT = sm.tile([128, _NB], f32, tag="qbT")      # [D, 32] block sums of q
    kbT = sm.tile([128, _NB], f32, tag="kbT")      # [D, 32] block sums of k

    vb3 = vb[:].rearrange("p (c x) -> p c x", x=129)

    # ---- stage A: load, transpose, pool ----
    for c in range(_NCHUNK):
        # k chunk
        k_nat = natp.tile([128, 128], f32, tag="nat")
        nc.sync.dma_start(k_nat[:], k_d[h, c * 128:(c + 1) * 128, :])
        tp = psM.tile([128, 128], f32, tag="m")
        nc.tensor.transpose(tp[:], k_nat[:], ident[:])
        # block sums (f32, exact) + bf16 copy of kT
        nc.vector.reduce_sum(
            kbT[:, 2 * c:2 * c + 2],
            tp[:].rearrange("p (b x) -> p b x", b=2),
            axis=AX.X,
        )
        nc.vector.tensor_copy(kT[:, c * 128:(c + 1) * 128], tp[:])

        # q chunk
        q_nat = natp.tile([128, 128], f32, tag="nat")
        nc.sync.dma_start(q_nat[:], q_d[h, c * 128:(c + 1) * 128, :])
        tq = psM.tile([128, 128], f32, tag="m")
        nc.tensor.transpose(tq[:], q_nat[:], ident[:])
        nc.vector.reduce_sum(
            qbT[:, 2 * c:2 * c + 2],
            tq[:].rearrange("p (b x) -> p b x", b=2),
            axis=AX.X,
        )
        nc.vector.tensor_copy(qT[:, c * 128:(c + 1) * 128], tq[:])

        # v chunk -> bf16 with ones column
        v_nat = natp.tile([128, 128], f32, tag="nat")
        nc.sync.dma_start(v_nat[:], v_d[h, c * 128:(c + 1) * 128, :])
        nc.vector.tensor_copy(vb3[:, c, 0:128], v_nat[:])
    nc.vector.memset(vb3[:, :, 128], 1.0)

    # ---- mean of k over S (smooth_k) ----
    ksum = sm.tile([128, 1], f32, tag="ksum")
    nc.vector.reduce_sum(ksum[:], kbT[:], axis=AX.X)
    mean64 = sm.tile([128, 1], f32, tag="mean64")   # 64 * mean
    nc.scalar.mul(mean64[:], ksum[:], 1.0 / float(_NB))
    mean_tok = sm.tile([128, 1], f32, tag="mean_tok")
    nc.scalar.mul(mean_tok[:], ksum[:], 1.0 / float(_S))
    # k_s = k - mean (in place, bf16); kb_s = kb_sum - 64*mean (f32)
    nc.vector.tensor_scalar_sub(kT[:], kT[:], mean_tok[:])
    kbs = sm.tile([128, _NB], f32, tag="kbs")
    nc.vector.tensor_scalar_sub(kbs[:], kbT[:], mean64[:])

    # ---- block scores & adaptive keep mask (f32, [32,32]) ----
    blp = psM.tile([32, 32], f32, tag="m")
    nc.tensor.matmul(blp[:], qbT[:], kbs[:], start=True, stop=True)
    bl = sm.tile([32, 32], f32, tag="bl")
    # bl = (qb.kb) * scale/(64*64) + causal(-1e30)
    nc.vector.scalar_tensor_tensor(
        bl[:], blp[:], _SCALE / float(_BLK * _BLK), causal_add[:],
        op0=OP.mult, op1=OP.add,
    )
    mx = sm.tile([32, 1], f32, tag="mx")
    nc.vector.reduce_max(mx[:], bl[:], axis=AX.X)
    nmx = sm.tile([32, 1], f32, tag="nmx")
    nc.vector.tensor_scalar_mul(nmx[:], mx[:], -1.0)
    bp = sm.tile([32, 32], f32, tag="bp")
    rs = sm.tile([32, 1], f32, tag="rs")
    nc.scalar.activation(bp[:], bl[:], AF.Exp, bias=nmx[:], scale=1.0,
                         accum_out=rs[:])
    rinv = sm.tile([32, 1], f32, tag="rinv")
    nc.vector.reciprocal(rinv[:], rs[:])
    nc.vector.tensor_scalar_mul(bp[:], bp[:], rinv[:])

    # T[i,j] = sum_j' bp[i,j'] * [bp[i,j'] > bp[i,j]]
    a_ap = bp[:].unsqueeze(1).broadcast_to((32, 32, 32))
    b_ap = bp[:].unsqueeze(2).broadcast_to((32, 32, 32))
    gt = sm.tile([32, 32 * 32], f32, tag="gt")
    gt3 = gt[:].rearrange("p (a b) -> p a b", a=32)
    nc.vector.tensor_tensor(gt3, a_ap, b_ap, op=OP.is_gt)
    pr = sm.tile([32, 32 * 32], f32, tag="pr")
    pr3 = pr[:].rearrange("p (a b) -> p a b", a=32)
    nc.vector.tensor_tensor(pr3, gt3, a_ap, op=OP.mult)
    tt = sm.tile([32, 32], f32, tag="tt")
    nc.vector.reduce_sum(tt[:], pr3, axis=AX.X)
    # keep = ((T < tau) & causal) | eye
    keep = sm.tile([32, 32], f32, tag="keep")
    nc.vector.scalar_tensor_tensor(
        keep[:], tt[:], _TAU, causal01[:], op0=OP.is_lt, op1=OP.mult)
    nc.vector.tensor_tensor(keep[:], keep[:], eye01[:], op=OP.max)
    keepT = sm.tile([32, 32], f32, tag="keepT")
    nc.vector.transpose(keepT[:], keep[:])
    # negk[kb, qi] = (keep[qb,kb]-1)*BIGM, expanded 64x along qi
    negk = sm.tile([32, _S], bf16, tag="negk")
    nc.vector.tensor_scalar(
        negk[:].rearrange("p (a b) -> p a b", b=_BLK),
        keepT[:].unsqueeze(2).broadcast_to((32, 32, _BLK)),
        1.0, _BIGM, op0=OP.subtract, op1=OP.mult,
    )

    # ---- main flash loop (transposed logits) ----
    for g in range(_NGRP):
        qlo = g * 512
        nchunks = 4 * g + 4  # causal kj chunks for this group
        # AV accumulators: one [128,129] psum bank per qi tile of the group
        acc_tiles = [psM.tile([128, 129], f32, tag="m", name=f"acc{g}_{i}")
                     for i in range(4)]
        accs = [a[:] for a in acc_tiles]
        for c0 in range(0, nchunks, 2):
            ltw = psP.tile([128, 1024], f32, tag="lt")
            ptw = ptp.tile([128, 1024], bf16, tag="pt")
            for ci in (c0, c0 + 1):
                sl = ltw[:, (ci - c0) * 512:(ci - c0) * 512 + 512]
                nc.tensor.matmul(
                    sl, kT[:, ci * 128:(ci + 1) * 128],
                    qT[:, qlo:qlo + 512], start=True, stop=False)
                nc.tensor.matmul(
                    sl, indall[:, ci * 128:(ci + 1) * 128],
                    negk[:, qlo:qlo + 512], start=False, stop=True)
                if ci * 128 >= qlo:  # diagonal tile: token-level causal
                    off = ci * 128 - qlo
                    nc.vector.tensor_tensor(
                        ltw[:, (ci - c0) * 512 + off:
                            (ci - c0) * 512 + off + 128],
                        ltw[:, (ci - c0) * 512 + off:
                            (ci - c0) * 512 + off + 128],
                        tri128[:], op=OP.add)
            nc.scalar.activation(ptw[:], ltw[:], AF.Exp,
                                 bias=nshift[:], scale=_SCALE)
            for ci in (c0, c0 + 1):
                for t in range(max(4 * g, ci), 4 * g + 4):
                    nc.tensor.matmul(
                        accs[t - 4 * g],
                        ptw[:, (ci - c0) * 512 + (t - 4 * g) * 128:
                            (ci - c0) * 512 + (t - 4 * g) * 128 + 128],
                        vb3[:, ci, :],
                        start=(ci == 0), stop=(ci == t))
        for t in range(4 * g, 4 * g + 4):
            acc = accs[t - 4 * g]
            rden = sm.tile([128, 1], f32, tag="rden")
            nc.vector.reciprocal(rden[:], acc[:, 128:129])
            o = outp.tile([128, 128], f32, tag="o")
            nc.vector.tensor_scalar_mul(o[:], acc[:, 0:128], rden[:])
            nc.sync.dma_start(out_d[h, t * 128:(t + 1) * 128, :], o[:])


def build_nc():
    import concourse.bass as bass
    import concourse.mybir as mybir
    import concourse.tile as tile
    from concourse import bacc
    from concourse.masks import make_identity

    f32 = mybir.dt.float32
    OP = mybir.AluOpType

    nc = bacc.Bacc("TRN2", target_bir_lowering=False, debug=False,
                   enable_asserts=False, num_devices=_NCORES)
    q_d = nc.dram_tensor("q", [_HLOC, _S, _D], f32, kind="ExternalInput").ap()
    k_d = nc.dram_tensor("k", [_HLOC, _S, _D], f32, kind="ExternalInput").ap()
    v_d = nc.dram_tensor("v", [_HLOC, _S, _D], f32, kind="ExternalInput").ap()
    out_d = nc.dram_tensor("out", [_HLOC, _S, _D], f32,
                           kind="ExternalOutput").ap()

    with tile.TileContext(nc) as tc:
        import contextlib
        with contextlib.ExitStack() as ctx:
            pools = {
                "natp": ctx.enter_context(tc.tile_pool(name="natp", bufs=8)),
                "psM": ctx.enter_context(
                    tc.tile_pool(name="psM", bufs=4, space="PSUM")),
                "psP": ctx.enter_context(
                    tc.tile_pool(name="psP", bufs=2, space="PSUM")),
                "big": ctx.enter_context(tc.tile_pool(name="big", bufs=2)),
                "sm": ctx.enter_context(tc.tile_pool(name="sm", bufs=2)),
                "ptp": ctx.enter_context(tc.tile_pool(name="ptp", bufs=3)),
                "outp": ctx.enter_context(tc.tile_pool(name="outp", bufs=4)),
                "constp": ctx.enter_context(
                    tc.tile_pool(name="constp", bufs=1)),
            }
            cp = pools["constp"]
            ident = cp.tile([128, 128], f32, tag="ident")
            make_identity(nc, ident[:])
            # indall[b, ci*128 + kj] = 1.0 iff b == 2*ci + kj//64
            indall = cp.tile([32, _NCHUNK * 128], mybir.dt.bfloat16,
                             tag="indall")
            nc.gpsimd.memset(indall[:], 1.0)
            nc.gpsimd.affine_select(
                out=indall[:], in_=indall[:], compare_op=OP.is_equal,
                fill=0.0, base=0,
                pattern=[[-2, _NCHUNK], [-1, 2], [0, _BLK]],
                channel_multiplier=1,
            )
            # tri128[p, f] = 0 if f >= p else -BIGM
            tri128 = cp.tile([128, 128], f32, tag="tri128")
            nc.gpsimd.memset(tri128[:], 0.0)
            nc.gpsimd.affine_select(
                out=tri128[:], in_=tri128[:], compare_op=OP.is_ge,
                fill=-_BIGM, base=0, pattern=[[1, 128]],
                channel_multiplier=-1,
            )
            # causal_add[32,32]: 0 lower incl diag, -1e30 above
            causal_add = cp.tile([32, 32], f32, tag="causal_add")
            nc.gpsimd.memset(causal_add[:], 0.0)
            nc.gpsimd.affine_select(
                out=causal_add[:], in_=causal_add[:], compare_op=OP.is_ge,
                fill=_NEG_BL, base=0, pattern=[[-1, 32]],
                channel_multiplier=1,
            )
            causal01 = cp.tile([32, 32], f32, tag="causal01")
            nc.gpsimd.memset(causal01[:], 1.0)
            nc.gpsimd.affine_select(
                out=causal01[:], in_=causal01[:], compare_op=OP.is_ge,
                fill=0.0, base=0, pattern=[[-1, 32]],
                channel_multiplier=1,
            )
            eye01 = cp.tile([32, 32], f32, tag="eye01")
            make_identity(nc, eye01[:])
            nshift = cp.tile([128, 1], f32, tag="nshift")
            nc.gpsimd.memset(nshift[:], -_SHIFT)
            consts = dict(ident=ident, indall=indall, tri128=tri128,
                          causal_add=causal_add, causal01=causal01,
                          eye01=eye01, nshift=nshift)
            for h in range(_HLOC):
                _build_head(nc, tc, pools, consts, q_d, k_d, v_d, out_d, h,
                            mybir, bass)
    nc.compile()
    return nc


_lock = threading.Lock()
_cached_nc = None


def _get_nc():
    global _cached_nc
    with _lock:
        if _cached_nc is None:
            _cached_nc = build_nc()
    return _cached_nc


def kernel(q, k, v):
    from concourse.bass_utils import run_bass_kernel_spmd

    q = np.asarray(q, dtype=np.float32)
    k = np.asarray(k, dtype=np.float32)
    v = np.asarray(v, dtype=np.float32)
    nc = _get_nc()
    in_maps = []
    for i in range(_NCORES):
        sl = slice(i * _HLOC, (i + 1) * _HLOC)
        in_maps.append({
            "q": np.ascontiguousarray(q[0, sl]),
            "k": np.ascontiguousarray(k[0, sl]),
            "v": np.ascontiguousarray(v[0, sl]),
        })
    res = run_bass_kernel_spmd(nc, in_maps, core_ids=list(range(_NCORES)))
    out = np.concatenate([res.results[i]["out"] for i in range(_NCORES)],
                         axis=0)
    return out.reshape(_B, _H, _S, _D)


if __name__ == "__main__":
    rng = np.random.default_rng(0)
    q = rng.standard_normal((_B, _H, _S, _D), dtype=np.float32)
    k = rng.standard_normal((_B, _H, _S, _D), dtype=np.float32)
    v = rng.standard_normal((_B, _H, _S, _D), dtype=np.float32)
    o = kernel(q, k, v)
    print(o.shape, o.dtype, np.abs(o).max())
